# revision 4
# baseline (speedup 1.0000x reference)
"""Trainium2 Bass kernel for EuclideanSimilarity (retrieval_knn).

Per-core pipeline (one batch element per NeuronCore, 8 cores):
  projT_m2[e,l] = (-2W)^T x + (-2b)      8 matmuls, stationary -2W^T
  qT[e,i]       = -0.25*(psum pooled)    pooled off the fp32 GEMM1 PSUM
  projnat[l,e]  = x_tile^T W^T + b       32 matmuls, xT tiles stationary
  ksq[j]        = sum_e projnat^2        one ACT Square+accum per tile
  qsq_bcast     = ones^T @ qT^2          reduce+partition-broadcast matmul
  per 512-query chunk (software-pipelined; GEMM2/GEMM3 operands f32r):
    psum  = -2 q.k ; power = psum + ksq[j] + qsq[i] (fused DVE)
    sim   = Exp(-Sqrt(power)) on ACT; kT += projnat_jt @ sim (GEMM3)
"""

import os
import sys

for _p in ("/opt/trn_rl_repo", "/root/.axon_site/_ro/trn_rl_repo"):
    if os.path.isdir(_p) and _p not in sys.path:
        sys.path.insert(0, _p)

import numpy as np

import concourse.bass as bass
import concourse.mybir as mybir
from concourse import bacc
from concourse.bass_utils import run_bass_kernel_spmd
from concourse.tile import TileContext
from concourse.tile_rust import add_dep_helper

B, L, D = 8, 4096, 128
LQ = L // 2
P = 128
NI = 512
NCHUNK = LQ // NI
NJT = L // P
F32 = mybir.dt.float32
F32R = mybir.dt.float32r

KMODE = os.environ.get("KMODE", "f32r")

AF = mybir.ActivationFunctionType
ALU = mybir.AluOpType


def build_nc(repeat=1, mode=None):
    mode = KMODE if mode is None else mode
    g2r = mode in ("f32r", "f32r2")
    g3r = mode == "f32r"
    G2DT = F32R if g2r else F32
    G3DT = F32R if g3r else F32
    nc = bacc.Bacc("TRN2", target_bir_lowering=False)

    xT = nc.declare_dram_parameter("xT", [P, L], F32R, isOutput=False)
    WT = nc.declare_dram_parameter("WT", [P, D], F32R, isOutput=False)
    Wm2T = nc.declare_dram_parameter("Wm2T", [P, D], F32R, isOutput=False)
    bcols = nc.declare_dram_parameter("bcols", [P, 2], F32, isOutput=False)
    b_bcast_in = nc.declare_dram_parameter("b_bcast", [P, D], F32, isOutput=False)
    ones_in = nc.declare_dram_parameter("ones_mat", [P, P], F32R, isOutput=False)

    qT_out = nc.declare_dram_parameter("qT", [P, LQ], F32, isOutput=True)
    kT_out = nc.declare_dram_parameter("kT", [P, LQ], F32, isOutput=True)

    with TileContext(nc) as tc:
      for _rep in range(repeat):
        with (
            tc.tile_pool(name="consts", bufs=1) as consts,
            tc.tile_pool(name="big", bufs=1) as big,
            tc.tile_pool(name="work", bufs=4) as work,
            tc.tile_pool(name="ps1", bufs=4, space="PSUM") as ps1,
        ):
            WT_sb = consts.tile([P, D], F32R)
            Wm2T_sb = consts.tile([P, D], F32R)
            bcols_sb = consts.tile([P, 2], F32)
            b_bcast = consts.tile([P, D], F32)
            ones_sb = consts.tile([P, P], F32R)
            nc.sync.dma_start(out=WT_sb[:], in_=WT[:])
            nc.sync.dma_start(out=Wm2T_sb[:], in_=Wm2T[:])
            nc.sync.dma_start(out=bcols_sb[:], in_=bcols[:])
            nc.sync.dma_start(out=b_bcast[:], in_=b_bcast_in[:])
            nc.sync.dma_start(out=ones_sb[:], in_=ones_in[:])
            b_col = bcols_sb[:, 0:1]
            bm2_col = bcols_sb[:, 1:2]

            projTm2 = big.tile([P, L], G2DT)
            projnat = big.tile([P, L], G3DT)
            if g2r:
                qT_mm = big.tile([P, LQ], G2DT, tag="qT_mm", name="qT_mm")
            qsq_bcast = big.tile([P, LQ], F32)
            ksq = consts.tile([P, NJT], F32)

            with tc.tile_pool(name="phase1", bufs=1) as ph1:
                xT_sb = ph1.tile([P, L], F32R)
                if g2r:
                    qT_sb = ph1.tile([P, LQ], F32, tag="qT_sb", name="qT_sb")
                else:
                    qT_sb = big.tile([P, LQ], F32, tag="qT_sb", name="qT_sb")
                    qT_mm = qT_sb
                for c in range(L // 512):
                    nc.sync.dma_start(
                        out=xT_sb[:, c * 512:(c + 1) * 512],
                        in_=xT[:, c * 512:(c + 1) * 512])

                for c in range(L // 512):
                    ps = ps1.tile([P, 512], F32, tag="ps1")
                    nc.tensor.matmul(
                        ps, Wm2T_sb[:], xT_sb[:, c * 512:(c + 1) * 512],
                        start=True, stop=True,
                    )
                    if g2r:
                        src32 = work.tile([P, 512], F32, tag="pm2f32")
                        nc.vector.tensor_scalar_add(src32[:], ps, bm2_col)
                        nc.vector.tensor_copy(
                            projTm2[:, c * 512:(c + 1) * 512], src32[:])
                        src32 = src32[:]
                    else:
                        src32 = projTm2[:, c * 512:(c + 1) * 512]
                        nc.vector.tensor_scalar_add(src32, ps, bm2_col)
                    sp = src32.rearrange("p (i two) -> p i two", two=2)
                    qtmp = work.tile([P, 256], F32, tag="qtmp")
                    nc.vector.tensor_add(qtmp[:], sp[:, :, 0], sp[:, :, 1])
                    nc.vector.tensor_scalar_mul(
                        qT_sb[:, c * 256:(c + 1) * 256], qtmp[:], -0.25)
                nc.sync.dma_start(out=qT_out[:], in_=qT_sb[:])
                if g2r:
                    nc.gpsimd.tensor_copy(qT_mm[:], qT_sb[:])

                for t in range(NJT):
                    ps = ps1.tile([P, D], F32, tag="ps1")
                    nc.tensor.matmul(
                        ps, xT_sb[:, t * P:(t + 1) * P], WT_sb[:],
                        start=True, stop=True,
                    )
                    if g3r:
                        seg32 = work.tile([P, D], F32, tag="sqs")
                        nc.vector.tensor_add(seg32[:], ps, b_bcast[:])
                        nc.vector.tensor_copy(
                            projnat[:, t * P:(t + 1) * P], seg32[:])
                    else:
                        seg32 = projnat[:, t * P:(t + 1) * P]
                        nc.vector.tensor_add(seg32, ps, b_bcast[:])
                    sq = work.tile([P, D], F32, tag="sqs")
                    nc.scalar.activation(
                        sq[:], seg32[:], AF.Square,
                        accum_out=ksq[:, t:t + 1])

                sq_qT = ph1.tile([P, LQ], F32R)
                nc.gpsimd.tensor_mul(sq_qT[:], qT_sb[:], qT_sb[:])
                for c in range(LQ // 512):
                    ps = ps1.tile([P, 512], F32, tag="ps1")
                    nc.tensor.matmul(
                        ps, ones_sb[:], sq_qT[:, c * 512:(c + 1) * 512],
                        start=True, stop=True,
                    )
                    nc.scalar.copy(qsq_bcast[:, c * 512:(c + 1) * 512], ps)

            NQ = 8
            QJT = NJT // NQ
            with (
                tc.tile_pool(name="stripp", bufs=NQ) as stripp,
                tc.tile_pool(name="simp", bufs=1) as simp,
                tc.tile_pool(name="psqk", bufs=3, space="PSUM") as psqk,
                tc.tile_pool(name="psk", bufs=1, space="PSUM") as psk,
            ):
                state = {}
                last_exp = {"i": None}

                def emit_power_sqrt(c):
                    qs = qsq_bcast[:, c * NI:(c + 1) * NI]
                    qchunk = qT_mm[:, c * NI:(c + 1) * NI]
                    sim = simp.tile([P, NJT * NI], G3DT, tag="sim", name="sim")
                    quarters = []
                    for h in range(NQ):
                        power = stripp.tile(
                            [P, QJT * NI], F32, tag="power", name="power")
                        quarters.append(power)
                        for j in range(QJT):
                            jt = h * QJT + j
                            ps2 = psqk.tile([P, NI], F32, tag="qk")
                            nc.tensor.matmul(
                                ps2, projTm2[:, jt * P:(jt + 1) * P], qchunk,
                                start=True, stop=True,
                            )
                            nc.vector.affine_then_add(
                                power[:, j * NI:(j + 1) * NI], ps2, qs,
                                scale=1.0, bias=ksq[:, jt:jt + 1],
                            )
                    sqrt_last = None
                    for h in range(NQ):
                        s = nc.scalar.activation(
                            quarters[h][:], quarters[h][:], AF.Sqrt)
                        prev = sqrt_last if h else last_exp["i"]
                        if prev is not None:
                            add_dep_helper(
                                s.ins, prev.ins, sync=False,
                                reason="act set batch: sqrt chain")
                        sqrt_last = s
                    state[c] = (quarters, sim, sqrt_last)

                def emit_exp_gemm3(c):
                    quarters, sim, sqrt_last = state.pop(c)
                    ps3 = psk.tile([P, NI], F32, tag="kacc")
                    for h in range(NQ):
                        e = nc.scalar.activation(
                            sim[:, h * QJT * NI:(h + 1) * QJT * NI],
                            quarters[h][:], AF.Exp, scale=-1.0)
                        prev = last_exp["i"] if h else sqrt_last
                        add_dep_helper(
                            e.ins, prev.ins, sync=False,
                            reason="act set batch: exp chain")
                        last_exp["i"] = e
                        for j in range(QJT):
                            jt = h * QJT + j
                            nc.tensor.matmul(
                                ps3, projnat[:, jt * P:(jt + 1) * P],
                                sim[:, jt * NI:(jt + 1) * NI],
                                start=(jt == 0), stop=(jt == NJT - 1),
                            )
                    kT_tile = work.tile([P, NI], F32, tag="kout")
                    nc.vector.tensor_copy(kT_tile[:], ps3)
                    nc.sync.dma_start(
                        out=kT_out[:, c * NI:(c + 1) * NI], in_=kT_tile[:])

                for c in range(NCHUNK):
                    if c >= 1:
                        emit_exp_gemm3(c - 1)
                    emit_power_sqrt(c)
                emit_exp_gemm3(NCHUNK - 1)

    nc.compile()
    return nc


def make_in_maps(x, W, b):
    x = np.asarray(x, np.float32)
    W = np.asarray(W, np.float32)
    b = np.asarray(b, np.float32)
    WT = np.ascontiguousarray(W.T)
    Wm2T = np.ascontiguousarray((-2.0 * W).T)
    bcols = np.stack([b, -2.0 * b], axis=1).astype(np.float32)
    b_bcast = np.ascontiguousarray(
        np.broadcast_to(b.reshape(1, D), (P, D)).astype(np.float32))
    ones_mat = np.ones((P, P), np.float32)
    return [{
        "xT": np.ascontiguousarray(x[i].T),
        "WT": WT, "Wm2T": Wm2T, "bcols": bcols, "b_bcast": b_bcast,
        "ones_mat": ones_mat,
    } for i in range(B)]


_NC_CACHE = {}


def _get_nc():
    key = ("nc", KMODE)
    if key not in _NC_CACHE:
        _NC_CACHE[key] = build_nc()
    return _NC_CACHE[key]


def kernel(x, W, b):
    nc = _get_nc()
    in_maps = make_in_maps(x, W, b)

    trace = bool(int(os.environ.get("KBENCH_TRACE", "0")))
    kres = None
    last_exc = None
    for attempt in range(5):
        try:
            kres = run_bass_kernel_spmd(nc, in_maps, list(range(B)), trace=trace)
            break
        except Exception as exc:
            last_exc = exc
            import time as _time
            _time.sleep(3.0 * (attempt + 1))
    if kres is None:
        raise last_exc
    _NC_CACHE["last_result"] = kres
    res = kres.results

    q = np.stack([np.ascontiguousarray(r["qT"].T) for r in res])
    k = np.stack([np.ascontiguousarray(r["kT"].T) for r in res])
    return q, k, k
